# revision 1
# baseline (speedup 1.0000x reference)
"""Trainium2 Bass kernel for nn_ConservativeDynamicCurvatureMLP.

Data-parallel over 8 NeuronCores: the batch (8192) is sharded into 8
local shards of 1024 rows; all weights are replicated. The curvature
scalar c_avg couples the shards through a global mean, handled with a
single-scalar AllReduce.

Math (reference):
    h = tanh(x @ W1 + b1)
    u = sigmoid(h @ W2 + b2)
    c = clip(mean(MIN_C + (MAX_C-MIN_C) * sigmoid(relu(x@cp_w1.T+cp_b1)@cp_w2.T+cp_b2)), MIN_C, MAX_C)
    z = poincare_ball_layer(h, u, c, T)
    out = z @ Wo + bo

The poincare layer collapses algebraically to z = alpha(row)*h + beta(row)*u
where alpha/beta are scalar functions of the row statistics
x2=||h||^2, y2=||u||^2, xy=<h,u> and c (verified to ~1e-6 against the
reference).  The NaN fallback (z <- h if any(isnan(z))) can only trigger when
den = 1 + 2c<x,y> + c^2 x2 y2 == -EPS exactly (measure-zero); it is omitted.

On-device layout is feature-major throughout: activations live as
[128 feature-partitions, kt, 1024 batch-cols] so every matmul consumes the
previous one's output directly (weights are the stationary operand in natural
[K, M] layout) and no transposes are needed.  Row statistics are computed with
a ones-vector matmul (partition reduction on the PE); the per-row scalar chain
runs batch-major on [128, 8] tiles via a small DRAM bounce.
"""

import tempfile
from contextlib import ExitStack

import numpy as np
import ml_dtypes

import concourse.bass as bass
import concourse.bacc as bacc
import concourse.mybir as mybir
import concourse.tile as tile
from concourse.bass_utils import run_bass_kernel_spmd

P = 128
N_CORES = 8
B_FULL = 8192
BL = B_FULL // N_CORES          # 1024 rows per core
IN = 3072
HID = 4096
OUT = 1000
KI = IN // P                    # 24
KH = HID // P                   # 32
NB = BL // P                    # 8 batch tiles
MIN_C = 0.001 * 0.5
MAX_C = 0.001 * 2.0
T_CONST = 0.7
EPS = 1e-7

dt = mybir.dt
AF = mybir.ActivationFunctionType
ALU = mybir.AluOpType
BF = ml_dtypes.bfloat16

_nc_cache = []


def _build(with_b1, with_b2):
    nc = bacc.Bacc("TRN2", target_bir_lowering=False, debug=False,
                   num_devices=N_CORES)

    xT_d = nc.dram_tensor("xT", [KI, P, BL], dt.bfloat16, kind="ExternalInput")
    # weight rows: w1r[mh, p, ki, q] = W1[ki*128+p, mh*128+q] -> one contiguous
    # 768KB DMA per output row-tile instead of 24 strided tile DMAs
    w1_d = nc.dram_tensor("w1", [KH, P, KI, P], dt.bfloat16, kind="ExternalInput")
    w2_d = nc.dram_tensor("w2", [KH, P, KH, P], dt.bfloat16, kind="ExternalInput")
    wo_d = nc.dram_tensor("wo", [KH, P, OUT], dt.bfloat16, kind="ExternalInput")
    cpw1_d = nc.dram_tensor("cpw1", [KI, P, 16], dt.bfloat16, kind="ExternalInput")
    cpw2_d = nc.dram_tensor("cpw2", [16, 1], dt.bfloat16, kind="ExternalInput")
    cpb1_d = nc.dram_tensor("cpb1", [16, 1], dt.float32, kind="ExternalInput")
    cpb2_d = nc.dram_tensor("cpb2", [1, 1], dt.float32, kind="ExternalInput")
    b1_d = nc.dram_tensor("b1", [P, KH], dt.float32, kind="ExternalInput") if with_b1 else None
    b2_d = nc.dram_tensor("b2", [P, KH], dt.float32, kind="ExternalInput") if with_b2 else None
    out_d = nc.dram_tensor("out", [BL, OUT], dt.float32, kind="ExternalOutput")

    f32 = dt.float32
    bf16 = dt.bfloat16

    with tile.TileContext(nc) as tc, ExitStack() as ctx:
        const = ctx.enter_context(tc.tile_pool(name="const", bufs=1))
        big = ctx.enter_context(tc.tile_pool(name="big", bufs=1))
        htp = ctx.enter_context(tc.tile_pool(name="htp", bufs=1))
        wp = ctx.enter_context(tc.tile_pool(name="wp", bufs=2))
        wop = ctx.enter_context(tc.tile_pool(name="wop", bufs=3))
        scr = ctx.enter_context(tc.tile_pool(name="scr", bufs=4))
        sacc = ctx.enter_context(tc.tile_pool(name="sacc", bufs=1))
        abp = ctx.enter_context(tc.tile_pool(name="abp", bufs=1))
        scal = ctx.enter_context(tc.tile_pool(name="scal", bufs=1))
        outp = ctx.enter_context(tc.tile_pool(name="outp", bufs=2))
        cpp = ctx.enter_context(tc.tile_pool(name="cpp", bufs=1))
        dram = ctx.enter_context(tc.tile_pool(name="dram", bufs=1, space="DRAM"))

        V = nc.vector
        S = nc.scalar

        def sc(name, shape=(P, 8), dtype=f32):
            return scal.tile(list(shape), dtype, name=name, tag=name)

        # ---------- persistent activations (feature-major) ----------
        # xT lands in 6 batched DMAs so MM1 can start consuming ki-group 0
        # while later groups stream; weight-row DMAs are emitted inside the
        # mh loops and interleave on the same queue.
        ones = const.tile([P, 1], f32, name="ones")
        nc.vector.memset(ones, 1.0)
        xT_sb = big.tile([P, KI, BL], bf16, name="xT_sb", tag="big",
                         padded_shape=[P, KH, BL])
        # first weight row issues on the sync queue before anything else;
        # xT streams via the otherwise-idle gpsimd queue in parallel
        w1row0 = wp.tile([P, KI, P], bf16, name="w1row", tag="w")
        nc.sync.dma_start(out=w1row0, in_=w1_d[0])
        for a, b in ((0, 2), (2, 4), (4, 8), (8, 12), (12, 16), (16, 20),
                     (20, 24)):
            nc.gpsimd.dma_start(
                out=xT_sb[:, a:b, :],
                in_=xT_d[a:b].rearrange("k p b -> p k b"))
        hT_sb = htp.tile([P, KH, BL], bf16, name="hT_sb")
        if with_b1:
            b1_sb = const.tile([P, KH], f32, name="b1_sb")
            nc.sync.dma_start(out=b1_sb, in_=b1_d[:, :])
        if with_b2:
            b2_sb = const.tile([P, KH], f32, name="b2_sb")
            nc.sync.dma_start(out=b2_sb, in_=b2_d[:, :])

        with ExitStack() as ph1:
            mm = ph1.enter_context(tc.tile_pool(name="mm", bufs=3, space="PSUM"))
            stp = ph1.enter_context(tc.tile_pool(name="stp", bufs=1, space="PSUM"))
            # stat rows live at partitions 0/32/64 (PSUM write base-partition
            # constraint): x2 @ 0, y2 @ 32, xy @ 64
            stat_ps = stp.tile([P, BL], f32, name="stat_ps")

            # ---------- MM1: hT = tanh(W1.T @ xT) , x2 stats ----------
            # Row statistics: the elementwise squares/products run on ACT/DVE
            # and accumulate (fp32, DVE) across mh; the PE only does one final
            # ones-matmul partition-reduction per stat per 512-chunk.
            x2a = sacc.tile([P, BL], f32, name="x2a")
            y2a = sacc.tile([P, BL], f32, name="y2a")
            xya = sacc.tile([P, BL], f32, name="xya")
            with nc.named_scope("mm1"):
                for mh in range(KH):
                    ps = mm.tile([P, BL], f32, name="ps", tag="mm")
                    if mh == 0:
                        w1row = w1row0
                    else:
                        w1row = wp.tile([P, KI, P], bf16, name="w1row",
                                        tag="w")
                        nc.sync.dma_start(out=w1row, in_=w1_d[mh])
                    for ki in range(KI):
                        nc.tensor.matmul(ps[:, 0:512], lhsT=w1row[:, ki, :],
                                         rhs=xT_sb[:, ki, 0:512],
                                         start=(ki == 0), stop=(ki == KI - 1))
                        nc.tensor.matmul(ps[:, 512:BL], lhsT=w1row[:, ki, :],
                                         rhs=xT_sb[:, ki, 512:BL],
                                         start=(ki == 0), stop=(ki == KI - 1))
                    if with_b1:
                        S.activation(hT_sb[:, mh, :], ps, AF.Tanh,
                                     bias=b1_sb[:, mh:mh + 1])
                    else:
                        S.activation(hT_sb[:, mh, :], ps, AF.Tanh)
                    hh = scr.tile([P, BL], bf16, name="hh", tag="hh")
                    S.activation(hh, hT_sb[:, mh, :], AF.Square)
                    if mh == 0:
                        V.tensor_copy(x2a, hh)
                    else:
                        V.tensor_add(x2a, x2a, hh)
                for ch in range(2):
                    sl = slice(ch * 512, (ch + 1) * 512)
                    nc.tensor.matmul(stat_ps[0:1, sl], lhsT=ones,
                                     rhs=x2a[:, sl], start=True, stop=True,
                                     skip_group_check=True)

            # ---------- curvature predictor (after MM1: xT still resident,
            # c is only needed after MM2, so the AllReduce hides easily) ----
            with nc.named_scope("cp"):
                cpw1_sb = const.tile([P, KI, 16], bf16, name="cpw1_sb")
                nc.sync.dma_start(out=cpw1_sb,
                                  in_=cpw1_d.rearrange("k p q -> p k q"))
                cpw2_sb = const.tile([16, 1], bf16, name="cpw2_sb")
                nc.sync.dma_start(out=cpw2_sb, in_=cpw2_d[:, :])
                cpb1_sb = const.tile([16, 1], f32, name="cpb1_sb")
                nc.sync.dma_start(out=cpb1_sb, in_=cpb1_d[:, :])
                cpb2_sb = const.tile([1, 1], f32, name="cpb2_sb")
                nc.sync.dma_start(out=cpb2_sb, in_=cpb2_d[:, :])
                cph_sb = cpp.tile([16, BL], bf16, name="cph_sb")
                for ch in range(2):
                    cps = mm.tile([16, 512], f32, name="cps", tag="mm")
                    for ki in range(KI):
                        nc.tensor.matmul(
                            cps, lhsT=cpw1_sb[:, ki, :],
                            rhs=xT_sb[:, ki, ch * 512:(ch + 1) * 512],
                            start=(ki == 0), stop=(ki == KI - 1))
                    S.activation(cph_sb[:, ch * 512:(ch + 1) * 512], cps,
                                 AF.Relu, bias=cpb1_sb)
                sparts = []
                for ch in range(2):
                    c2p = mm.tile([1, 512], f32, name="c2p", tag="mm")
                    nc.tensor.matmul(c2p, lhsT=cpw2_sb,
                                     rhs=cph_sb[:16, ch * 512:(ch + 1) * 512],
                                     start=True, stop=True)
                    cpw = cpp.tile([1, 512], f32, name="cpw", tag="cpw")
                    spart = cpp.tile([1, 1], f32, name=f"spart{ch}",
                                     tag=f"spart{ch}")
                    S.activation(cpw, c2p, AF.Sigmoid, bias=cpb2_sb,
                                 accum_out=spart)
                    sparts.append(spart)
                s_loc = cpp.tile([1, 1], f32, name="s_loc")
                V.tensor_add(s_loc, sparts[0], sparts[1])
                cin = dram.tile([1, 1], f32, name="cin")
                cout = dram.tile([1, 1], f32, name="cout")
                nc.sync.dma_start(out=cin, in_=s_loc)
                nc.gpsimd.collective_compute(
                    "AllReduce", ALU.add,
                    replica_groups=[list(range(N_CORES))],
                    ins=[cin.opt()], outs=[cout.opt()])
                s_b = sc("s_b", (P, 1))
                nc.gpsimd.dma_start(out=s_b, in_=cout.to_broadcast([P, 1]))
                # c = clip(MIN_C + (MAX_C-MIN_C)*mean(c_pred))
                c_b = sc("c_b", (P, 1))
                V.tensor_scalar(out=c_b, in0=s_b,
                                scalar1=(MAX_C - MIN_C) / B_FULL,
                                scalar2=MIN_C, op0=ALU.mult, op1=ALU.add)
                V.tensor_scalar_min(out=c_b, in0=c_b, scalar1=MAX_C)
                V.tensor_scalar_max(out=c_b, in0=c_b, scalar1=MIN_C)
                negc_b = sc("negc_b", (P, 1))
                V.tensor_scalar_mul(out=negc_b, in0=c_b, scalar1=-1.0)
                twoc_b = sc("twoc_b", (P, 1))
                V.tensor_scalar_mul(out=twoc_b, in0=c_b, scalar1=2.0)
                neg2c_b = sc("neg2c_b", (P, 1))
                V.tensor_scalar_mul(out=neg2c_b, in0=c_b, scalar1=-2.0)
                c2_b = sc("c2_b", (P, 1))
                V.tensor_mul(c2_b, c_b, c_b)

            # ---------- MM2: uT = sigmoid(W2.T @ hT) , y2/xy stats ----------
            uT_sb = big.tile([P, KH, BL], bf16, name="uT_sb", tag="big")
            with nc.named_scope("mm2"):
                for mh in range(KH):
                    ps = mm.tile([P, BL], f32, name="ps", tag="mm")
                    w2row = wp.tile([P, KH, P], bf16, name="w2row", tag="w")
                    nc.sync.dma_start(out=w2row, in_=w2_d[mh])
                    for kh in range(KH):
                        nc.tensor.matmul(ps[:, 0:512], lhsT=w2row[:, kh, :],
                                         rhs=hT_sb[:, kh, 0:512],
                                         start=(kh == 0), stop=(kh == KH - 1))
                        nc.tensor.matmul(ps[:, 512:BL], lhsT=w2row[:, kh, :],
                                         rhs=hT_sb[:, kh, 512:BL],
                                         start=(kh == 0), stop=(kh == KH - 1))
                    if with_b2:
                        S.activation(uT_sb[:, mh, :], ps, AF.Sigmoid,
                                     bias=b2_sb[:, mh:mh + 1])
                    else:
                        S.activation(uT_sb[:, mh, :], ps, AF.Sigmoid)
                    uu = scr.tile([P, BL], bf16, name="uu", tag="hh")
                    S.activation(uu, uT_sb[:, mh, :], AF.Square)
                    hu = scr.tile([P, BL], bf16, name="hu", tag="hh")
                    V.tensor_mul(hu, hT_sb[:, mh, :], uT_sb[:, mh, :])
                    if mh == 0:
                        V.tensor_copy(y2a, uu)
                        V.tensor_copy(xya, hu)
                    else:
                        V.tensor_add(y2a, y2a, uu)
                        V.tensor_add(xya, xya, hu)
                for ch in range(2):
                    sl = slice(ch * 512, (ch + 1) * 512)
                    nc.tensor.matmul(stat_ps[32:33, sl], lhsT=ones,
                                     rhs=y2a[:, sl], start=True, stop=True,
                                     skip_group_check=True)
                    nc.tensor.matmul(stat_ps[64:65, sl], lhsT=ones,
                                     rhs=xya[:, sl], start=True, stop=True,
                                     skip_group_check=True)

            # ---------- stats -> batch-major, per column-half ----------
            # split by 512-column half so the half-0 scalar chain (which
            # gates zcomb0/mmo0) starts without waiting for half-1 plumbing
            with nc.named_scope("stats"):
                stats_sb = scal.tile([P, BL], f32, name="stats_sb", tag="stats_sb")
                st_d = dram.tile([3, BL], f32, name="st_d")
                for ch in range(2):
                    hsl = slice(ch * 512, (ch + 1) * 512)
                    for i, r in enumerate((0, 32, 64)):
                        S.copy(stats_sb[r:r + 1, hsl], stat_ps[r:r + 1, hsl])
                        nc.sync.dma_start(out=st_d[i, hsl],
                                          in_=stats_sb[r:r + 1, hsl])

        # psum pools (mm, stp) released here

        # ---------- per-row scalar chain (batch-major [128, 4] per half) ---
        alpha_b = abp.tile([P, BL], f32, name="alpha_b")
        beta_b = abp.tile([P, BL], f32, name="beta_b")
        ab_d = dram.tile([2, BL], f32, name="ab_d")

        def scalar_chain(ch):
            hsl = slice(ch * 512, (ch + 1) * 512)

            def sch(name):
                return sc(f"{name}_{ch}", (P, 4))

            x2 = sch("x2")
            y2 = sch("y2")
            xy = sch("xy")
            for i, t in enumerate((x2, y2, xy)):
                nc.sync.dma_start(
                    out=t, in_=st_d[i, hsl].rearrange("(j p) -> p j", p=P))
            w = sch("w")
            V.scalar_tensor_tensor(out=w, in0=xy, scalar=-2.0, in1=y2,
                                   op0=ALU.mult, op1=ALU.add)
            A1 = sch("A1")
            V.tensor_scalar(out=A1, in0=w, scalar1=c_b, scalar2=1.0,
                            op0=ALU.mult, op1=ALU.add)
            A2 = sch("A2")
            V.tensor_scalar(out=A2, in0=x2, scalar1=negc_b, scalar2=1.0,
                            op0=ALU.mult, op1=ALU.add)
            p1 = sch("p1")
            V.tensor_mul(p1, x2, y2)
            den = sch("den")
            V.tensor_scalar(out=den, in0=p1, scalar1=c2_b, scalar2=1.0,
                            op0=ALU.mult, op1=ALU.add)
            V.scalar_tensor_tensor(out=den, in0=xy, scalar=neg2c_b, in1=den,
                                   op0=ALU.mult, op1=ALU.add)
            V.tensor_scalar_add(out=den, in0=den, scalar1=EPS)
            D = sch("D")
            V.reciprocal(D, den)
            # ||a||^2 = D^2 (A1^2 x2 - 2 A1 A2 xy + A2^2 y2)
            t1 = sch("t1")
            V.tensor_mul(t1, A1, A1)
            V.tensor_mul(t1, t1, x2)
            t2 = sch("t2")
            V.tensor_mul(t2, A1, A2)
            V.tensor_mul(t2, t2, xy)
            t3 = sch("t3")
            V.tensor_mul(t3, A2, A2)
            V.tensor_mul(t3, t3, y2)
            na2 = sch("na2")
            V.scalar_tensor_tensor(out=na2, in0=t2, scalar=-2.0, in1=t1,
                                   op0=ALU.mult, op1=ALU.add)
            V.tensor_add(na2, na2, t3)
            dsq = sch("dsq")
            V.tensor_mul(dsq, D, D)
            V.tensor_mul(na2, na2, dsq)
            # q = sqrt(c * na2) with one Newton step (ACT sqrt is low precision)
            q2 = sch("q2")
            V.tensor_scalar(out=q2, in0=na2, scalar1=c_b, scalar2=None,
                            op0=ALU.mult)
            q0 = sch("q0")
            S.activation(q0, q2, AF.Sqrt)
            V.tensor_scalar_max(out=q0, in0=q0, scalar1=1e-20)
            r0 = sch("r0")
            V.reciprocal(r0, q0)
            q = sch("q")
            V.tensor_mul(q, q2, r0)
            V.tensor_add(q, q, q0)
            V.tensor_scalar_mul(out=q, in0=q, scalar1=0.5)
            arg = sch("arg")
            V.tensor_scalar_min(out=arg, in0=q, scalar1=1.0 - 1e-5)
            # artanh(arg) = 0.5 ln((1+arg)/(1-arg)); t = tanh(T*artanh)/q
            opp = sch("opp")
            V.tensor_scalar(out=opp, in0=arg, scalar1=-1.0, scalar2=1.0,
                            op0=ALU.mult, op1=ALU.add)
            opn = sch("opn")
            V.tensor_scalar_add(out=opn, in0=arg, scalar1=1.0)
            rr = sch("rr")
            V.reciprocal(rr, opp)
            rat = sch("rat")
            V.tensor_mul(rat, opn, rr)
            lg = sch("lg")
            S.activation(lg, rat, AF.Ln)
            th = sch("th")
            S.activation(th, lg, AF.Tanh, scale=T_CONST * 0.5)
            rq = sch("rq")
            V.reciprocal(rq, q)
            tm = sch("tm")
            V.tensor_mul(tm, th, rq)
            # <h,a> = D (A2 xy - A1 x2)
            s1_ = sch("s1_")
            V.tensor_mul(s1_, A1, x2)
            s2_ = sch("s2_")
            V.tensor_mul(s2_, A2, xy)
            ha = sch("ha")
            V.tensor_sub(ha, s2_, s1_)
            V.tensor_mul(ha, ha, D)
            hm = sch("hm")
            V.tensor_mul(hm, tm, ha)
            tsq = sch("tsq")
            V.tensor_mul(tsq, tm, tm)
            m2 = sch("m2")
            V.tensor_mul(m2, tsq, na2)
            w2s = sch("w2s")
            V.scalar_tensor_tensor(out=w2s, in0=hm, scalar=2.0, in1=m2,
                                   op0=ALU.mult, op1=ALU.add)
            B1 = sch("B1")
            V.tensor_scalar(out=B1, in0=w2s, scalar1=c_b, scalar2=1.0,
                            op0=ALU.mult, op1=ALU.add)
            p2 = sch("p2")
            V.tensor_mul(p2, x2, m2)
            den2 = sch("den2")
            V.tensor_scalar(out=den2, in0=p2, scalar1=c2_b, scalar2=1.0,
                            op0=ALU.mult, op1=ALU.add)
            V.scalar_tensor_tensor(out=den2, in0=hm, scalar=twoc_b, in1=den2,
                                   op0=ALU.mult, op1=ALU.add)
            V.tensor_scalar_add(out=den2, in0=den2, scalar1=EPS)
            D2 = sch("D2")
            V.reciprocal(D2, den2)
            g = sch("g")
            V.tensor_mul(g, A2, tm)
            V.tensor_mul(g, g, D)
            w3 = sch("w3")
            V.tensor_mul(w3, g, A1)
            V.tensor_sub(w3, B1, w3)
            alpha_bm = sch("alpha_bm")
            V.tensor_mul(alpha_bm, w3, D2)
            w4 = sch("w4")
            V.tensor_mul(w4, g, A2)
            beta_bm = sch("beta_bm")
            V.tensor_mul(beta_bm, w4, D2)
            # bounce to DRAM in batch-linear order, broadcast back
            nc.sync.dma_start(
                out=ab_d[0, hsl].rearrange("(j p) -> p j", p=P), in_=alpha_bm)
            nc.sync.dma_start(
                out=ab_d[1, hsl].rearrange("(j p) -> p j", p=P), in_=beta_bm)
            nc.gpsimd.dma_start(out=alpha_b[:, hsl],
                                in_=ab_d[0:1, hsl].to_broadcast([P, 512]))
            nc.gpsimd.dma_start(out=beta_b[:, hsl],
                                in_=ab_d[1:2, hsl].to_broadcast([P, 512]))

        with nc.named_scope("scalars"):
            scalar_chain(0)
            scalar_chain(1)

        # ---------- z = alpha*h + beta*u (overwrites uT in place),
        # then out = z @ Wo.  Processed in two batch-column halves so the
        # MMo matmuls of half 0 overlap the DVE z-combine of half 1.
        with ExitStack() as ph2:
            mmo = ph2.enter_context(tc.tile_pool(name="mmo", bufs=8,
                                                 space="PSUM"))
            for bg in range(2):
                csl = slice(bg * 512, (bg + 1) * 512)
                with nc.named_scope(f"zcomb{bg}"):
                    for kh in range(KH):
                        t1z = scr.tile([P, 512], bf16, name="t1z", tag="zz",
                                       bufs=4)
                        V.tensor_mul(t1z, hT_sb[:, kh, csl], alpha_b[:, csl])
                        t2z = scr.tile([P, 512], bf16, name="t2z", tag="zz",
                                       bufs=4)
                        V.tensor_mul(t2z, uT_sb[:, kh, csl], beta_b[:, csl])
                        V.tensor_add(uT_sb[:, kh, csl], t1z, t2z)
                with nc.named_scope(f"mmo{bg}"):
                    pso = [mmo.tile([P, 500], f32, name=f"pso{bg}_{i}",
                                    tag="mmo") for i in range(8)]
                    for kh in range(KH):
                        wot = wop.tile([P, OUT], bf16, name="wot", tag="wo")
                        nc.sync.dma_start(out=wot, in_=wo_d[kh])
                        for i in range(4):
                            b = bg * 4 + i
                            for och in range(2):
                                nc.tensor.matmul(
                                    pso[i * 2 + och],
                                    lhsT=uT_sb[:, kh, b * P:(b + 1) * P],
                                    rhs=wot[:, och * 500:(och + 1) * 500],
                                    start=(kh == 0), stop=(kh == KH - 1))
                    for i in range(4):
                        b = bg * 4 + i
                        ob = outp.tile([P, OUT], f32, name="ob", tag="ob")
                        S.copy(ob[:, 0:500], pso[i * 2])
                        V.tensor_copy(ob[:, 500:OUT], pso[i * 2 + 1])
                        nc.sync.dma_start(out=out_d[b * P:(b + 1) * P, :],
                                          in_=ob)

    nc.compile()
    return nc


def _get_nc(with_b1, with_b2):
    for k, v in _nc_cache:
        if k == (with_b1, with_b2):
            return v
    nc = _build(with_b1, with_b2)
    _nc_cache.append(((with_b1, with_b2), nc))
    return nc


def kernel(x, W1, b1, W2, b2, Wo, bo, cp_w1, cp_b1, cp_w2, cp_b2,
           _trace=False, _tmpdir=None):
    x = np.asarray(x, dtype=np.float32)
    with_b1 = bool(np.any(b1))
    with_b2 = bool(np.any(b2))
    nc = _get_nc(with_b1, with_b2)

    # w1r[mh, p, ki, q] = W1[ki*128+p, mh*128+q]
    w1_t = np.ascontiguousarray(
        np.asarray(W1, np.float32).reshape(KI, P, KH, P).transpose(2, 1, 0, 3)
    ).astype(BF)
    w2_t = np.ascontiguousarray(
        np.asarray(W2, np.float32).reshape(KH, P, KH, P).transpose(2, 1, 0, 3)
    ).astype(BF)
    wo_t = np.asarray(Wo, np.float32).reshape(KH, P, OUT).astype(BF)
    cpw1_t = np.ascontiguousarray(
        np.asarray(cp_w1, np.float32).T.reshape(KI, P, 16)).astype(BF)
    cpw2_t = np.asarray(cp_w2, np.float32).reshape(1, 16).T.astype(BF)
    cpw2_t = np.ascontiguousarray(cpw2_t)
    cpb1_t = np.asarray(cp_b1, np.float32).reshape(16, 1)
    cpb2_t = np.asarray(cp_b2, np.float32).reshape(1, 1)
    b1_t = np.ascontiguousarray(np.asarray(b1, np.float32).reshape(KH, P).T)
    b2_t = np.ascontiguousarray(np.asarray(b2, np.float32).reshape(KH, P).T)

    in_maps = []
    for c in range(N_CORES):
        shard = x[c * BL:(c + 1) * BL]
        xT = np.ascontiguousarray(shard.T).reshape(KI, P, BL).astype(BF)
        m = {"xT": xT, "w1": w1_t, "w2": w2_t, "wo": wo_t,
             "cpw1": cpw1_t, "cpw2": cpw2_t, "cpb1": cpb1_t, "cpb2": cpb2_t}
        if with_b1:
            m["b1"] = b1_t
        if with_b2:
            m["b2"] = b2_t
        in_maps.append(m)

    kw = {}
    if _trace:
        kw = dict(trace=True, tmpdir=_tmpdir or tempfile.mkdtemp(prefix="cdk_"))
    res = run_bass_kernel_spmd(nc, in_maps, list(range(N_CORES)), **kw)

    out = np.concatenate([res.results[c]["out"] for c in range(N_CORES)], axis=0)
    bo = np.asarray(bo, np.float32)
    if np.any(bo):
        out = out + bo
    if _trace:
        kernel._last_result = res
    return out



# revision 5
# speedup vs baseline: 1.2435x; 1.2435x over previous
"""Trainium2 Bass kernel for nn_ConservativeDynamicCurvatureMLP.

Data-parallel over 8 NeuronCores: batch (8192) sharded into 8 shards of
1024 rows; weights replicated.  The curvature scalar couples shards via a
single-scalar AllReduce.

Math (reference):
    h = tanh(x @ W1 + b1)
    u = sigmoid(h @ W2 + b2)
    c = clip(mean(MIN_C + (MAX_C-MIN_C) * sigmoid(relu(x@cp_w1.T+cp_b1)@cp_w2.T+cp_b2)), MIN_C, MAX_C)
    z = poincare_ball_layer(h, u, c, T)   ==  alpha(row)*h + beta(row)*u
    out = z @ Wo + bo

Performance structure (v2):
  * MM1 (x@W1) in bf16, feature-major, as before.
  * MM2 (h@W2) in fp8 e4m3 with DoubleRow perf mode: 2 k-slices per
    matmul -> 2x PE throughput.  h is cast bf16->e4m3 on DVE (direct cast,
    |h|<=1), W2 is host-prescaled by 256 into e4m3 (max |W2*256| ~ 28 << 240);
    the 1/256 is folded into the sigmoid activation scale.  Predicted
    end-to-end rel err ~1.3e-2 (gate 2e-2), simulated with exact RNE casts.
  * MM2 runs column-chunk-major (two 512-column halves): the per-half
    stats -> scalar-chain -> z-combine tail overlaps the other half's
    matmuls / the output projection, keeping the PE dense (no HAM
    re-throttle).
  * MMo (z@Wo) in bf16, och-wave structure, overlapped with the half-1
    z-combine.
"""

import tempfile
from contextlib import ExitStack

import numpy as np
import ml_dtypes

import concourse.bass as bass
import concourse.bacc as bacc
import concourse.mybir as mybir
import concourse.tile as tile
from concourse.bass_utils import run_bass_kernel_spmd

P = 128
N_CORES = 8
B_FULL = 8192
BL = B_FULL // N_CORES          # 1024 rows per core
IN = 3072
HID = 4096
OUT = 1000
KI = IN // P                    # 24
KH = HID // P                   # 32
KP = KH // 2                    # 16 DoubleRow k-pairs
MIN_C = 0.001 * 0.5
MAX_C = 0.001 * 2.0
T_CONST = 0.7
EPS = 1e-7
W2_SCALE = 256.0

dt = mybir.dt
AF = mybir.ActivationFunctionType
ALU = mybir.AluOpType
DR = mybir.MatmulPerfMode.DoubleRow
BF = ml_dtypes.bfloat16
E4 = ml_dtypes.float8_e4m3fn

_nc_cache = []


def _build(with_b1, with_b2):
    nc = bacc.Bacc("TRN2", target_bir_lowering=False, debug=False,
                   num_devices=N_CORES)

    xT_d = nc.dram_tensor("xT", [KI, P, BL], dt.bfloat16, kind="ExternalInput")
    # w1r[mh, p, ki, q] = W1[ki*128+p, mh*128+q]
    w1_d = nc.dram_tensor("w1", [KH, P, KI, P], dt.bfloat16, kind="ExternalInput")
    # w2r[mh, p, kp, j, q] = W2[(2*kp+j)*128+p, mh*128+q] * 256  (fp8 pairs)
    w2_d = nc.dram_tensor("w2", [KH, P, KP, 2, P], dt.float8e4,
                          kind="ExternalInput")
    wo_d = nc.dram_tensor("wo", [KH, P, OUT], dt.bfloat16, kind="ExternalInput")
    cpw1_d = nc.dram_tensor("cpw1", [KI, P, 16], dt.bfloat16, kind="ExternalInput")
    cpw2_d = nc.dram_tensor("cpw2", [16, 1], dt.bfloat16, kind="ExternalInput")
    cpb1_d = nc.dram_tensor("cpb1", [16, 1], dt.float32, kind="ExternalInput")
    cpb2_d = nc.dram_tensor("cpb2", [1, 1], dt.float32, kind="ExternalInput")
    b1_d = nc.dram_tensor("b1", [P, KH], dt.float32, kind="ExternalInput") if with_b1 else None
    b2_d = nc.dram_tensor("b2", [P, KH], dt.float32, kind="ExternalInput") if with_b2 else None
    out_d = nc.dram_tensor("out", [BL, OUT], dt.float32, kind="ExternalOutput")

    f32 = dt.float32
    bf16 = dt.bfloat16
    fp8 = dt.float8e4

    with tile.TileContext(nc) as tc, ExitStack() as ctx:
        const = ctx.enter_context(tc.tile_pool(name="const", bufs=1))
        big = ctx.enter_context(tc.tile_pool(name="big", bufs=1))
        htp = ctx.enter_context(tc.tile_pool(name="htp", bufs=1))
        hqp = ctx.enter_context(tc.tile_pool(name="hqp", bufs=1))
        wp = ctx.enter_context(tc.tile_pool(name="wp", bufs=2))
        wop = ctx.enter_context(tc.tile_pool(name="wop", bufs=3))
        scr = ctx.enter_context(tc.tile_pool(name="scr", bufs=3))
        zscr = ctx.enter_context(tc.tile_pool(name="zscr", bufs=2))
        sacc = ctx.enter_context(tc.tile_pool(name="sacc", bufs=2))
        abp = ctx.enter_context(tc.tile_pool(name="abp", bufs=1))
        scal = ctx.enter_context(tc.tile_pool(name="scal", bufs=1))
        outp = ctx.enter_context(tc.tile_pool(name="outp", bufs=2))
        cpp = ctx.enter_context(tc.tile_pool(name="cpp", bufs=1))
        dram = ctx.enter_context(tc.tile_pool(name="dram", bufs=1, space="DRAM"))

        V = nc.vector
        S = nc.scalar

        def sc(name, shape=(P, 8), dtype=f32):
            return scal.tile(list(shape), dtype, name=name, tag=name)

        # ---------- persistent activations (feature-major) ----------
        ones = const.tile([P, 1], f32, name="ones")
        nc.vector.memset(ones, 1.0)
        xT_sb = big.tile([P, KI, BL], bf16, name="xT_sb", tag="big",
                         padded_shape=[P, KH, BL])
        w1row0 = wp.tile([P, KI, P], bf16, name="w1row", tag="w")
        nc.sync.dma_start(out=w1row0, in_=w1_d[0])
        for a, b in ((0, 2), (2, 4), (4, 8), (8, 12), (12, 16), (16, 20),
                     (20, 24)):
            nc.gpsimd.dma_start(
                out=xT_sb[:, a:b, :],
                in_=xT_d[a:b].rearrange("k p b -> p k b"))
        hT_sb = htp.tile([P, KH, BL], bf16, name="hT_sb")
        hq_sb = hqp.tile([P, KP, 2, BL], fp8, name="hq_sb")
        if with_b1:
            b1_sb = const.tile([P, KH], f32, name="b1_sb")
            nc.sync.dma_start(out=b1_sb, in_=b1_d[:, :])
        if with_b2:
            b2_sb = const.tile([P, KH], f32, name="b2_sb")
            nc.sync.dma_start(out=b2_sb, in_=b2_d[:, :])

        st_d = dram.tile([3, BL], f32, name="st_d")
        ab_d = dram.tile([2, BL], f32, name="ab_d")
        alpha_b = abp.tile([P, BL], bf16, name="alpha_b")
        beta_b = abp.tile([P, BL], bf16, name="beta_b")

        with ExitStack() as ph1:
            mm = ph1.enter_context(tc.tile_pool(name="mm", bufs=2, space="PSUM"))
            stp = ph1.enter_context(tc.tile_pool(name="stp", bufs=2, space="PSUM"))
            # per-half stat psums; rows: x2 @ 0, y2 @ 32, xy @ 64
            stat_ps = [stp.tile([P, 512], f32, name=f"stat_ps{ch}",
                                tag="stat") for ch in range(2)]

            # ---------- MM1: hT = tanh(W1.T @ xT), hq cast, x2 stats ------
            x2a = sacc.tile([P, BL], f32, name="x2a", tag="sacc")
            with nc.named_scope("mm1"):
                for mh in range(KH):
                    ps = mm.tile([P, BL], f32, name="ps", tag="mm")
                    if mh == 0:
                        w1row = w1row0
                    else:
                        w1row = wp.tile([P, KI, P], bf16, name="w1row",
                                        tag="w")
                        nc.sync.dma_start(out=w1row, in_=w1_d[mh])
                    for ki in range(KI):
                        nc.tensor.matmul(ps[:, 0:512], lhsT=w1row[:, ki, :],
                                         rhs=xT_sb[:, ki, 0:512],
                                         start=(ki == 0), stop=(ki == KI - 1))
                        nc.tensor.matmul(ps[:, 512:BL], lhsT=w1row[:, ki, :],
                                         rhs=xT_sb[:, ki, 512:BL],
                                         start=(ki == 0), stop=(ki == KI - 1))
                    if with_b1:
                        S.activation(hT_sb[:, mh, :], ps, AF.Tanh,
                                     bias=b1_sb[:, mh:mh + 1])
                    else:
                        S.activation(hT_sb[:, mh, :], ps, AF.Tanh)
                    V.tensor_copy(hq_sb[:, mh // 2, mh % 2, :],
                                  hT_sb[:, mh, :])
                    hh = scr.tile([P, BL], bf16, name="hh", tag="hh")
                    S.activation(hh, hT_sb[:, mh, :], AF.Square)
                    if mh == 0:
                        V.tensor_copy(x2a, hh)
                    else:
                        V.tensor_add(x2a, x2a, hh)
                for ch in range(2):
                    sl = slice(ch * 512, (ch + 1) * 512)
                    nc.tensor.matmul(stat_ps[ch][0:1, :], lhsT=ones,
                                     rhs=x2a[:, sl], start=True, stop=True,
                                     skip_group_check=True)

            # ---------- curvature predictor + AllReduce ----------
            with nc.named_scope("cp"):
                cpw1_sb = const.tile([P, KI, 16], bf16, name="cpw1_sb")
                nc.sync.dma_start(out=cpw1_sb,
                                  in_=cpw1_d.rearrange("k p q -> p k q"))
                cpw2_sb = const.tile([16, 1], bf16, name="cpw2_sb")
                nc.sync.dma_start(out=cpw2_sb, in_=cpw2_d[:, :])
                cpb1_sb = const.tile([16, 1], f32, name="cpb1_sb")
                nc.sync.dma_start(out=cpb1_sb, in_=cpb1_d[:, :])
                cpb2_sb = const.tile([1, 1], f32, name="cpb2_sb")
                nc.sync.dma_start(out=cpb2_sb, in_=cpb2_d[:, :])
                cph_sb = cpp.tile([16, BL], fp8, name="cph_sb")
                for ch in range(2):
                    cps = mm.tile([16, 512], f32, name="cps", tag="mm")
                    for ki in range(KI):
                        nc.tensor.matmul(
                            cps, lhsT=cpw1_sb[:, ki, :],
                            rhs=xT_sb[:, ki, ch * 512:(ch + 1) * 512],
                            start=(ki == 0), stop=(ki == KI - 1))
                    S.activation(cph_sb[:, ch * 512:(ch + 1) * 512], cps,
                                 AF.Relu, bias=cpb1_sb)
                sparts = []
                for ch in range(2):
                    c2p = mm.tile([1, 512], f32, name="c2p", tag="mm")
                    nc.tensor.matmul(c2p, lhsT=cpw2_sb,
                                     rhs=cph_sb[:16, ch * 512:(ch + 1) * 512],
                                     start=True, stop=True)
                    cpw = cpp.tile([1, 512], bf16, name="cpw", tag="cpw")
                    spart = cpp.tile([1, 1], f32, name=f"spart{ch}",
                                     tag=f"spart{ch}")
                    S.activation(cpw, c2p, AF.Sigmoid, bias=cpb2_sb,
                                 accum_out=spart)
                    sparts.append(spart)
                s_loc = cpp.tile([1, 1], f32, name="s_loc")
                V.tensor_add(s_loc, sparts[0], sparts[1])
                cin = dram.tile([1, 1], f32, name="cin")
                cout = dram.tile([1, 1], f32, name="cout")
                nc.sync.dma_start(out=cin, in_=s_loc)
                nc.gpsimd.collective_compute(
                    "AllReduce", ALU.add,
                    replica_groups=[list(range(N_CORES))],
                    ins=[cin.opt()], outs=[cout.opt()])
                s_b = sc("s_b", (P, 1))
                nc.gpsimd.dma_start(out=s_b, in_=cout.to_broadcast([P, 1]))
                c_b = sc("c_b", (P, 1))
                V.tensor_scalar(out=c_b, in0=s_b,
                                scalar1=(MAX_C - MIN_C) / B_FULL,
                                scalar2=MIN_C, op0=ALU.mult, op1=ALU.add)
                V.tensor_scalar_min(out=c_b, in0=c_b, scalar1=MAX_C)
                V.tensor_scalar_max(out=c_b, in0=c_b, scalar1=MIN_C)
                negc_b = sc("negc_b", (P, 1))
                V.tensor_scalar_mul(out=negc_b, in0=c_b, scalar1=-1.0)
                twoc_b = sc("twoc_b", (P, 1))
                V.tensor_scalar_mul(out=twoc_b, in0=c_b, scalar1=2.0)
                neg2c_b = sc("neg2c_b", (P, 1))
                V.tensor_scalar_mul(out=neg2c_b, in0=c_b, scalar1=-2.0)
                c2_b = sc("c2_b", (P, 1))
                V.tensor_mul(c2_b, c_b, c_b)

        # still inside ph1 scope vars; reopened below for MM2
            # ---------- per-row scalar chain (batch-major [128, 4]) -------
            y2a = sacc.tile([P, BL], f32, name="y2a", tag="sacc")
            xya = sacc.tile([P, BL], f32, name="xya", tag="sacc")
            uT_sb = big.tile([P, KH, BL], bf16, name="uT_sb", tag="big")

            def scalar_chain(ch):
                hsl = slice(ch * 512, (ch + 1) * 512)

                def sch(name):
                    return sc(f"{name}_{ch}", (P, 4))

                x2 = sch("x2")
                y2 = sch("y2")
                xy = sch("xy")
                for i, t in enumerate((x2, y2, xy)):
                    nc.sync.dma_start(
                        out=t, in_=st_d[i, hsl].rearrange("(j p) -> p j", p=P))
                w = sch("w")
                V.scalar_tensor_tensor(out=w, in0=xy, scalar=-2.0, in1=y2,
                                       op0=ALU.mult, op1=ALU.add)
                A1 = sch("A1")
                V.tensor_scalar(out=A1, in0=w, scalar1=c_b, scalar2=1.0,
                                op0=ALU.mult, op1=ALU.add)
                A2 = sch("A2")
                V.tensor_scalar(out=A2, in0=x2, scalar1=negc_b, scalar2=1.0,
                                op0=ALU.mult, op1=ALU.add)
                p1 = sch("p1")
                V.tensor_mul(p1, x2, y2)
                den = sch("den")
                V.tensor_scalar(out=den, in0=p1, scalar1=c2_b, scalar2=1.0,
                                op0=ALU.mult, op1=ALU.add)
                V.scalar_tensor_tensor(out=den, in0=xy, scalar=neg2c_b, in1=den,
                                       op0=ALU.mult, op1=ALU.add)
                V.tensor_scalar_add(out=den, in0=den, scalar1=EPS)
                D = sch("D")
                V.reciprocal(D, den)
                t1 = sch("t1")
                V.tensor_mul(t1, A1, A1)
                V.tensor_mul(t1, t1, x2)
                t2 = sch("t2")
                V.tensor_mul(t2, A1, A2)
                V.tensor_mul(t2, t2, xy)
                t3 = sch("t3")
                V.tensor_mul(t3, A2, A2)
                V.tensor_mul(t3, t3, y2)
                na2 = sch("na2")
                V.scalar_tensor_tensor(out=na2, in0=t2, scalar=-2.0, in1=t1,
                                       op0=ALU.mult, op1=ALU.add)
                V.tensor_add(na2, na2, t3)
                dsq = sch("dsq")
                V.tensor_mul(dsq, D, D)
                V.tensor_mul(na2, na2, dsq)
                q2 = sch("q2")
                V.tensor_scalar(out=q2, in0=na2, scalar1=c_b, scalar2=None,
                                op0=ALU.mult)
                q0 = sch("q0")
                S.activation(q0, q2, AF.Sqrt)
                V.tensor_scalar_max(out=q0, in0=q0, scalar1=1e-20)
                r0 = sch("r0")
                V.reciprocal(r0, q0)
                q = sch("q")
                V.tensor_mul(q, q2, r0)
                V.tensor_add(q, q, q0)
                V.tensor_scalar_mul(out=q, in0=q, scalar1=0.5)
                arg = sch("arg")
                V.tensor_scalar_min(out=arg, in0=q, scalar1=1.0 - 1e-5)
                opp = sch("opp")
                V.tensor_scalar(out=opp, in0=arg, scalar1=-1.0, scalar2=1.0,
                                op0=ALU.mult, op1=ALU.add)
                opn = sch("opn")
                V.tensor_scalar_add(out=opn, in0=arg, scalar1=1.0)
                rr = sch("rr")
                V.reciprocal(rr, opp)
                rat = sch("rat")
                V.tensor_mul(rat, opn, rr)
                lg = sch("lg")
                S.activation(lg, rat, AF.Ln)
                th = sch("th")
                S.activation(th, lg, AF.Tanh, scale=T_CONST * 0.5)
                rq = sch("rq")
                V.reciprocal(rq, q)
                tm = sch("tm")
                V.tensor_mul(tm, th, rq)
                s1_ = sch("s1_")
                V.tensor_mul(s1_, A1, x2)
                s2_ = sch("s2_")
                V.tensor_mul(s2_, A2, xy)
                ha = sch("ha")
                V.tensor_sub(ha, s2_, s1_)
                V.tensor_mul(ha, ha, D)
                hm = sch("hm")
                V.tensor_mul(hm, tm, ha)
                tsq = sch("tsq")
                V.tensor_mul(tsq, tm, tm)
                m2 = sch("m2")
                V.tensor_mul(m2, tsq, na2)
                w2s = sch("w2s")
                V.scalar_tensor_tensor(out=w2s, in0=hm, scalar=2.0, in1=m2,
                                       op0=ALU.mult, op1=ALU.add)
                B1 = sch("B1")
                V.tensor_scalar(out=B1, in0=w2s, scalar1=c_b, scalar2=1.0,
                                op0=ALU.mult, op1=ALU.add)
                p2 = sch("p2")
                V.tensor_mul(p2, x2, m2)
                den2 = sch("den2")
                V.tensor_scalar(out=den2, in0=p2, scalar1=c2_b, scalar2=1.0,
                                op0=ALU.mult, op1=ALU.add)
                V.scalar_tensor_tensor(out=den2, in0=hm, scalar=twoc_b, in1=den2,
                                       op0=ALU.mult, op1=ALU.add)
                V.tensor_scalar_add(out=den2, in0=den2, scalar1=EPS)
                D2 = sch("D2")
                V.reciprocal(D2, den2)
                g = sch("g")
                V.tensor_mul(g, A2, tm)
                V.tensor_mul(g, g, D)
                w3 = sch("w3")
                V.tensor_mul(w3, g, A1)
                V.tensor_sub(w3, B1, w3)
                alpha_bm = sch("alpha_bm")
                V.tensor_mul(alpha_bm, w3, D2)
                w4 = sch("w4")
                V.tensor_mul(w4, g, A2)
                beta_bm = sch("beta_bm")
                V.tensor_mul(beta_bm, w4, D2)
                nc.sync.dma_start(
                    out=ab_d[0, hsl].rearrange("(j p) -> p j", p=P),
                    in_=alpha_bm)
                nc.sync.dma_start(
                    out=ab_d[1, hsl].rearrange("(j p) -> p j", p=P),
                    in_=beta_bm)
                nc.gpsimd.dma_start(out=alpha_b[:, hsl],
                                    in_=ab_d[0:1, hsl].to_broadcast([P, 512]))
                nc.gpsimd.dma_start(out=beta_b[:, hsl],
                                    in_=ab_d[1:2, hsl].to_broadcast([P, 512]))

            def mm2_mh(ch, mh):
                csl = slice(ch * 512, (ch + 1) * 512)
                ps = mm.tile([P, 512], f32, name="ps2", tag="mm")
                w2row = wp.tile([P, KP, 2, P], fp8, name="w2row", tag="w")
                nc.sync.dma_start(out=w2row, in_=w2_d[mh])
                for kp in range(KP):
                    nc.tensor.matmul(ps, lhsT=w2row[:, kp],
                                     rhs=hq_sb[:, kp, :, csl],
                                     start=(kp == 0), stop=(kp == KP - 1),
                                     perf_mode=DR)
                if with_b2:
                    S.activation(uT_sb[:, mh, csl], ps, AF.Sigmoid,
                                 bias=b2_sb[:, mh:mh + 1],
                                 scale=1.0 / W2_SCALE)
                else:
                    S.activation(uT_sb[:, mh, csl], ps, AF.Sigmoid,
                                 scale=1.0 / W2_SCALE)
                uu = scr.tile([P, 512], bf16, name="uu", tag="hh")
                S.activation(uu, uT_sb[:, mh, csl], AF.Square)
                hu = scr.tile([P, 512], bf16, name="hu", tag="hh")
                V.tensor_mul(hu, hT_sb[:, mh, csl], uT_sb[:, mh, csl])
                if mh == 0:
                    V.tensor_copy(y2a[:, csl], uu)
                    V.tensor_copy(xya[:, csl], hu)
                else:
                    V.tensor_add(y2a[:, csl], y2a[:, csl], uu)
                    V.tensor_add(xya[:, csl], xya[:, csl], hu)

            def mm2_stats(ch):
                csl = slice(ch * 512, (ch + 1) * 512)
                nc.tensor.matmul(stat_ps[ch][32:33, :], lhsT=ones,
                                 rhs=y2a[:, csl], start=True, stop=True,
                                 skip_group_check=True)
                nc.tensor.matmul(stat_ps[ch][64:65, :], lhsT=ones,
                                 rhs=xya[:, csl], start=True, stop=True,
                                 skip_group_check=True)
                hsl = slice(ch * 512, (ch + 1) * 512)
                stats_sb = scal.tile([P, 512], f32, name=f"stats_sb{ch}",
                                     tag="stats_sb")
                for i, r in enumerate((0, 32, 64)):
                    S.copy(stats_sb[r:r + 1, :], stat_ps[ch][r:r + 1, :])
                    nc.sync.dma_start(out=st_d[i, hsl],
                                      in_=stats_sb[r:r + 1, :])

            def zcomb(ch):
                csl = slice(ch * 512, (ch + 1) * 512)
                for kh in range(KH):
                    t1z = zscr.tile([P, 512], bf16, name="t1z", tag="zz")
                    V.tensor_mul(t1z, hT_sb[:, kh, csl], alpha_b[:, csl])
                    t2z = zscr.tile([P, 512], bf16, name="t2z", tag="zz")
                    V.tensor_mul(t2z, uT_sb[:, kh, csl], beta_b[:, csl])
                    V.tensor_add(uT_sb[:, kh, csl], t1z, t2z)

            # ---------- MM2 ch0 ----------
            with nc.named_scope("mm2a"):
                for mh in range(KH):
                    mm2_mh(0, mh)
                mm2_stats(0)
            # ---------- MM2 ch1, chain0 interleaved after 2 mh ----------
            with nc.named_scope("mm2b"):
                mm2_mh(1, 0)
                mm2_mh(1, 1)
                with nc.named_scope("chain0"):
                    scalar_chain(0)
                for mh in range(2, 18):
                    mm2_mh(1, mh)
                with nc.named_scope("zcomb0"):
                    zcomb(0)
                for mh in range(18, KH):
                    mm2_mh(1, mh)
                mm2_stats(1)
            with nc.named_scope("chain1"):
                scalar_chain(1)
        # ph1 psum pools (mm, stp) released here

        # ---------- MMo: out = z @ Wo, och waves; zcomb1 under mmo0 ------
        with ExitStack() as ph2:
            mmo = ph2.enter_context(tc.tile_pool(name="mmo", bufs=4,
                                                 space="PSUM"))

            def mmo_ch(ch):
                for och in range(2):
                    osl = slice(och * 500, (och + 1) * 500)
                    pso = [mmo.tile([P, 500], f32, name=f"pso{ch}_{och}_{i}",
                                    tag="mmo") for i in range(4)]
                    for kh in range(KH):
                        wot = wop.tile([P, 500], bf16, name="wot", tag="wo")
                        nc.sync.dma_start(out=wot, in_=wo_d[kh][:, osl])
                        for i in range(4):
                            b = ch * 4 + i
                            nc.tensor.matmul(
                                pso[i],
                                lhsT=uT_sb[:, kh, b * P:(b + 1) * P],
                                rhs=wot,
                                start=(kh == 0), stop=(kh == KH - 1))
                    for i in range(4):
                        b = ch * 4 + i
                        ob = outp.tile([P, 500], f32, name="ob", tag="ob")
                        if och == 0:
                            S.copy(ob, pso[i])
                        else:
                            V.tensor_copy(ob, pso[i])
                        nc.sync.dma_start(
                            out=out_d[b * P:(b + 1) * P, osl], in_=ob)

            with nc.named_scope("zcomb1"):
                zcomb(1)
            with nc.named_scope("mmo0"):
                mmo_ch(0)
            with nc.named_scope("mmo1"):
                mmo_ch(1)

    nc.compile()
    return nc


def _get_nc(with_b1, with_b2):
    for k, v in _nc_cache:
        if k == (with_b1, with_b2):
            return v
    nc = _build(with_b1, with_b2)
    _nc_cache.append(((with_b1, with_b2), nc))
    return nc


def kernel(x, W1, b1, W2, b2, Wo, bo, cp_w1, cp_b1, cp_w2, cp_b2,
           _trace=False, _tmpdir=None):
    x = np.asarray(x, dtype=np.float32)
    with_b1 = bool(np.any(b1))
    with_b2 = bool(np.any(b2))
    nc = _get_nc(with_b1, with_b2)

    # w1r[mh, p, ki, q] = W1[ki*128+p, mh*128+q]
    w1_t = np.ascontiguousarray(
        np.asarray(W1, np.float32).reshape(KI, P, KH, P).transpose(2, 1, 0, 3)
    ).astype(BF)
    # w2r[mh, p, kp, j, q] = W2[(2*kp+j)*128+p, mh*128+q] * 256 in e4m3
    w2_t = np.ascontiguousarray(
        (np.asarray(W2, np.float32) * np.float32(W2_SCALE))
        .reshape(KP, 2, P, KH, P).transpose(3, 2, 0, 1, 4)
    ).astype(E4)
    wo_t = np.asarray(Wo, np.float32).reshape(KH, P, OUT).astype(BF)
    cpw1_t = np.ascontiguousarray(
        np.asarray(cp_w1, np.float32).T.reshape(KI, P, 16)).astype(BF)
    cpw2_t = np.asarray(cp_w2, np.float32).reshape(1, 16).T.astype(BF)
    cpw2_t = np.ascontiguousarray(cpw2_t)
    cpb1_t = np.asarray(cp_b1, np.float32).reshape(16, 1)
    cpb2_t = np.asarray(cp_b2, np.float32).reshape(1, 1)
    b1_t = np.ascontiguousarray(np.asarray(b1, np.float32).reshape(KH, P).T)
    b2_t = np.ascontiguousarray(np.asarray(b2, np.float32).reshape(KH, P).T)

    in_maps = []
    for c in range(N_CORES):
        shard = x[c * BL:(c + 1) * BL]
        xT = np.ascontiguousarray(shard.T).reshape(KI, P, BL).astype(BF)
        m = {"xT": xT, "w1": w1_t, "w2": w2_t, "wo": wo_t,
             "cpw1": cpw1_t, "cpw2": cpw2_t, "cpb1": cpb1_t, "cpb2": cpb2_t}
        if with_b1:
            m["b1"] = b1_t
        if with_b2:
            m["b2"] = b2_t
        in_maps.append(m)

    kw = {}
    if _trace:
        kw = dict(trace=True, tmpdir=_tmpdir or tempfile.mkdtemp(prefix="cdk_"))
    res = run_bass_kernel_spmd(nc, in_maps, list(range(N_CORES)), **kw)

    out = np.concatenate([res.results[c]["out"] for c in range(N_CORES)], axis=0)
    bo = np.asarray(bo, np.float32)
    if np.any(bo):
        out = out + bo
    if _trace:
        kernel._last_result = res
    return out
